# revision 2
# baseline (speedup 1.0000x reference)
"""AttnBlock kernel for 8 Trainium2 NeuronCores — single-pass fp16 variant.

Problem: x[4,512,64,64] f32 -> GroupNorm(2 groups, eps 1e-6) -> q,k,v 1x1 convs
-> attention over N=4096 positions with scale sqrt(512) (multiplied) -> proj
-> residual.

Sharding: 8 cores = 4 examples x 2 query-halves (columns rotated per core so
its half comes first; softmax over keys is permutation invariant). No
cross-core communication.

Precision: single-pass fp16 q/k convs and score matmuls (validated on the
fixed inputs: rel_err ~8.7e-3 < 2e-2 gate). GN in fp32; value path fp16;
softmax online over two key halves with fp32 PSUM scores.
"""

import math

import numpy as np

import concourse.bacc as bacc
import concourse.mybir as mybir
import concourse.tile as tile
from concourse.bass_utils import run_bass_kernel_spmd
from concourse.masks import make_identity

F32 = mybir.dt.float32
F16 = mybir.dt.float16

B, C, H, W = 4, 512, 64, 64
N = H * W            # 4096 key positions
NQ = N // 2          # 2048 query positions per core
P = 128              # partitions
CT = C // P          # 4 channel tiles
NCH = N // 512       # 8 key chunks of 512
NQB = NQ // P        # 16 query blocks of 128
G = 2                # groupnorm groups
EPS = 1e-6
AX = mybir.AxisListType.X
ALU = mybir.AluOpType
ACTF = mybir.ActivationFunctionType

_CACHED_NC = None


def build_nc(loop_r: int = 1):
    nc = bacc.Bacc("TRN2", target_bir_lowering=False)

    x_d = nc.dram_tensor("x", [CT, P, N], F32, kind="ExternalInput")
    wqt_d = nc.dram_tensor("wqt", [P, CT, C], F16, kind="ExternalInput")  # [p, t, o], q scaled by sqrt(C)
    wkt_d = nc.dram_tensor("wkt", [P, CT, C], F16, kind="ExternalInput")
    wvt_d = nc.dram_tensor("wvt", [P, CT, C], F16, kind="ExternalInput")
    wpt_d = nc.dram_tensor("wpt", [P, CT, C], F16, kind="ExternalInput")
    # per-channel params packed: [p, t, (bq, bk, bp, gnw, gnb, pad)]
    prm_d = nc.dram_tensor("prm", [P, CT, 6], F32, kind="ExternalInput")
    bv_d = nc.dram_tensor("bv", [1, C], F16, kind="ExternalInput")        # row layout
    out_d = nc.dram_tensor("out", [CT, P, NQ], F32, kind="ExternalOutput")

    import contextlib

    with tile.TileContext(nc) as tc:
        loop_ctx = tc.For_i(0, loop_r, 1) if loop_r > 1 else contextlib.nullcontext()
        with (
            loop_ctx,
            tc.tile_pool(name="singles", bufs=1) as singles,
            tc.tile_pool(name="persist", bufs=1) as persist,
            tc.tile_pool(name="xs_pool", bufs=3) as xs_pool,
        ):
            ident = singles.tile([P, P], F16, name="ident")
            make_identity(nc, ident)
            ones_f32 = singles.tile([P, P], F32, name="ones_f32")
            nc.vector.memset(ones_f32, 1.0)
            ones_bf = singles.tile([1, P], F16, name="ones_bf")
            nc.vector.memset(ones_bf, 1.0)
            inv256 = singles.tile([P, 1], F32, name="inv256")
            nc.vector.memset(inv256, 1.0 / 256.0)
            eps_t = singles.tile([P, 1], F32, name="eps_t")
            nc.vector.memset(eps_t, EPS)

            # weights and per-channel params: one packed DMA each
            wqt_all = persist.tile([P, CT, C], F16, name="wqt_all")
            wkt_all = persist.tile([P, CT, C], F16, name="wkt_all")
            wvt_all = persist.tile([P, CT, C], F16, name="wvt_all")
            wpt_all = persist.tile([P, CT, C], F16, name="wpt_all")
            prm = persist.tile([P, CT, 6], F32, name="prm")
            bv_row = persist.tile([1, C], F16, name="bv_row")
            nc.gpsimd.dma_start(out=wqt_all, in_=wqt_d[:, :, :])
            nc.gpsimd.dma_start(out=wkt_all, in_=wkt_d[:, :, :])
            nc.gpsimd.dma_start(out=wvt_all, in_=wvt_d[:, :, :])
            nc.gpsimd.dma_start(out=wpt_all, in_=wpt_d[:, :, :])
            nc.gpsimd.dma_start(out=prm, in_=prm_d[:, :, :])
            nc.gpsimd.dma_start(out=bv_row, in_=bv_d[:, :])
            wqt = [wqt_all[:, t, :] for t in range(CT)]
            wkt = [wkt_all[:, t, :] for t in range(CT)]
            wvt = [wvt_all[:, t, :] for t in range(CT)]
            wpt = [wpt_all[:, t, :] for t in range(CT)]
            bq = [prm[:, t, 0:1] for t in range(CT)]
            bk = [prm[:, t, 1:2] for t in range(CT)]
            bp = [prm[:, t, 2:3] for t in range(CT)]
            gnw = [prm[:, t, 3:4] for t in range(CT)]
            gnb = [prm[:, t, 4:5] for t in range(CT)]

            # persistent activations (single-pass fp16)
            k16 = [persist.tile([P, N], F16, name=f"k16_{t}") for t in range(CT)]
            q16 = [persist.tile([P, NQ], F16, name=f"q16_{t}") for t in range(CT)]
            vTa = persist.tile([P, N // P, C], F16, name="vTa")
            vT = [vTa[:, m, :] for m in range(N // P)]
            out_ca = persist.tile([P, CT, NQ], F16, name="out_ca")
            out_c = [out_ca[:, t, :] for t in range(CT)]

            # ---------------- Phase 1: GroupNorm statistics ----------------
            with (
                tc.tile_pool(name="stat_sb", bufs=1) as stat_sb,
                tc.tile_pool(name="stat_ps", bufs=2, space="PSUM") as stat_ps,
            ):
                stats6 = [stat_sb.tile([P, NCH, 6], F32, name=f"st6_{t}") for t in range(CT)]
                for t in range(CT):
                    for hf in range(2):
                        xb = stat_sb.tile([P, N // 2], F32, name="xbig", tag="xbig", bufs=3)
                        nc.sync.dma_start(
                            out=xb, in_=x_d[t][:, hf * (N // 2):(hf + 1) * (N // 2)])
                        for c2 in range(NCH // 2):
                            ch = hf * (NCH // 2) + c2
                            nc.vector.bn_stats(
                                out=stats6[t][:, ch, :], in_=xb[:, c2 * 512:(c2 + 1) * 512])
                mvs = stat_sb.tile([P, CT, 2], F32, name="mvs")
                for t in range(CT):
                    nc.vector.bn_aggr(out=mvs[:, t, :], in_=stats6[t])
                # stats2 cols: [mean_t0..3 | ex2_t0..3]
                stats2 = stat_sb.tile([P, 8], F32, name="stats2")
                means = mvs[:, :, 0]
                vars_ = mvs[:, :, 1]
                nc.vector.tensor_copy(stats2[:, 0:4], means)
                nc.vector.tensor_tensor(out=stats2[:, 4:8], in0=means, in1=means, op=ALU.mult)
                nc.vector.tensor_tensor(out=stats2[:, 4:8], in0=stats2[:, 4:8], in1=vars_, op=ALU.add)
                # column sums / 256 -> [1, 8] on partition 0
                ps8 = stat_ps.tile([1, 8], F32, name="ps8")
                nc.tensor.matmul(ps8, inv256, stats2, start=True, stop=True)
                s8 = stat_sb.tile([1, 8], F32, name="s8")
                nc.vector.tensor_copy(s8, ps8)
                # per-group mean and E[x^2]: adjacent-pair sums
                gme = stat_sb.tile([1, 4], F32, name="gme")  # [mu_g0, mu_g1, e_g0, e_g1]
                s8v = s8.rearrange("p (f g two) -> p f g two", f=2, two=2)
                gmev = gme.rearrange("p (f g) -> p f g", f=2)
                nc.vector.tensor_tensor(
                    out=gmev[:, :, :], in0=s8v[:, :, :, 0], in1=s8v[:, :, :, 1], op=ALU.add)
                # broadcast to 128 partitions: [128, 4]
                psb = stat_ps.tile([P, 4], F32, name="psb")
                nc.tensor.matmul(psb, ones_f32[0:1, :], gme, start=True, stop=True)
                mu_e = stat_sb.tile([P, 4], F32, name="mu_e")
                nc.vector.tensor_copy(mu_e, psb)
                mu_bc = mu_e[:, 0:2]
                e_bc = mu_e[:, 2:4]
                var_bc = stat_sb.tile([P, 2], F32, name="var_bc")
                nc.vector.tensor_tensor(out=var_bc, in0=mu_bc, in1=mu_bc, op=ALU.mult)
                nc.vector.tensor_tensor(out=var_bc, in0=e_bc, in1=var_bc, op=ALU.subtract)
                sd = stat_sb.tile([P, 2], F32, name="sd")
                for g in range(G):
                    nc.scalar.activation(out=sd[:, g:g + 1], in_=var_bc[:, g:g + 1],
                                         func=ACTF.Sqrt, bias=eps_t, scale=1.0)
                rstd = stat_sb.tile([P, 2], F32, name="rstd")
                nc.vector.reciprocal(out=rstd, in_=sd)
                # per-channel-tile affine: h = a*x + b
                a_t = [persist.tile([P, 1], F32, name=f"a_t{t}") for t in range(CT)]
                b_t = [persist.tile([P, 1], F32, name=f"b_t{t}") for t in range(CT)]
                for t in range(CT):
                    g = t // 2
                    nc.vector.tensor_tensor(
                        out=a_t[t], in0=gnw[t], in1=rstd[:, g:g + 1], op=ALU.mult)
                    nc.vector.tensor_tensor(
                        out=b_t[t], in0=mu_bc[:, g:g + 1], in1=a_t[t], op=ALU.mult)
                    nc.vector.tensor_tensor(
                        out=b_t[t], in0=gnb[t], in1=b_t[t], op=ALU.subtract)

            # ---------------- Phase 2: h + q/k/v convs (streamed) ----------------
            with (
                tc.tile_pool(name="h16_pool", bufs=8) as h16_pool,
                tc.tile_pool(name="cq_ps", bufs=2, space="PSUM") as cq_ps,
                tc.tile_pool(name="ck_ps", bufs=2, space="PSUM") as ck_ps,
                tc.tile_pool(name="cv_ps", bufs=2, space="PSUM") as cv_ps,
            ):
                for ch in range(NCH):
                    sl = slice(ch * 512, (ch + 1) * 512)
                    h16 = []
                    for t in range(CT):
                        xs = xs_pool.tile([P, 512], F32, name="xs2", tag="xs")
                        nc.gpsimd.dma_start(out=xs, in_=x_d[t][:, sl])
                        h16t = h16_pool.tile([P, 512], F16, name="h16", tag="h16")
                        nc.vector.tensor_scalar(
                            out=h16t, in0=xs, scalar1=a_t[t], scalar2=b_t[t],
                            op0=ALU.mult, op1=ALU.add)
                        h16.append(h16t)
                    # k conv (and q for first half): single-pass fp16
                    for o in range(CT):
                        osl = slice(o * P, (o + 1) * P)
                        kp = ck_ps.tile([P, 512], F32, name="kp", tag="kp")
                        for t in range(CT):
                            nc.tensor.matmul(
                                kp, wkt[t][:, osl], h16[t],
                                start=(t == 0), stop=(t == CT - 1))
                        nc.scalar.activation(
                            out=k16[o][:, sl], in_=kp, func=ACTF.Identity,
                            bias=bk[o], scale=1.0)
                        if ch < NCH // 2:
                            qp = cq_ps.tile([P, 512], F32, name="qp", tag="qp")
                            for t in range(CT):
                                nc.tensor.matmul(
                                    qp, wqt[t][:, osl], h16[t],
                                    start=(t == 0), stop=(t == CT - 1))
                            nc.scalar.activation(
                                out=q16[o][:, sl], in_=qp, func=ACTF.Identity,
                                bias=bq[o], scale=1.0)
                    # v conv, transposed output
                    for mb in range(4):
                        m = ch * 4 + mb
                        vp = cv_ps.tile([P, C], F32, name="vp", tag="vp")
                        for t in range(CT):
                            nc.tensor.matmul(
                                vp, h16[t][:, mb * P:(mb + 1) * P], wvt[t],
                                start=(t == 0), stop=False)
                        nc.tensor.matmul(vp, ones_bf, bv_row, start=False, stop=True)
                        nc.vector.tensor_copy(vT[m], vp)

            # ---------------- Phase 3: attention ----------------
            with (
                tc.tile_pool(name="att_sb", bufs=1) as att_sb,
                tc.tile_pool(name="p_pool", bufs=2) as p_pool,
                tc.tile_pool(name="pt_pool", bufs=2) as pt_pool,
                tc.tile_pool(name="ot_pool", bufs=2) as ot_pool,
                tc.tile_pool(name="sc_ps", bufs=2, space="PSUM") as sc_ps,
                tc.tile_pool(name="tp_ps", bufs=2, space="PSUM") as tp_ps,
                tc.tile_pool(name="o_ps", bufs=1, space="PSUM") as o_ps,
                tc.tile_pool(name="pp_ps", bufs=1, space="PSUM") as pp_ps,
                tc.tile_pool(name="fin_pool", bufs=3) as fin_pool,
            ):
                def emit_proj(nch):
                    sl = slice(nch * 512, (nch + 1) * 512)
                    for o in range(CT):
                        pp = pp_ps.tile([P, 512], F32, name="pp", tag="pp")
                        for t in range(CT):
                            nc.tensor.matmul(
                                pp, wpt[t][:, o * P:(o + 1) * P], out_c[t][:, sl],
                                start=(t == 0), stop=(t == CT - 1))
                        fin = fin_pool.tile([P, 512], F32, name="fin", tag="fin")
                        nc.scalar.activation(
                            out=fin, in_=pp, func=ACTF.Identity, bias=bp[o], scale=1.0)
                        xr = xs_pool.tile([P, 512], F32, name="xr", tag="xs")
                        nc.gpsimd.dma_start(out=xr, in_=x_d[o][:, sl])
                        nc.vector.tensor_tensor(out=fin, in0=fin, in1=xr, op=ALU.add)
                        nc.gpsimd.dma_start(out=out_d[o][:, sl], in_=fin)

                def emit_scores_a(nb):
                    """Pass A: scores chunks 0-3 + their softmax stats."""
                    pt_b = p_pool.tile([P, N], F16, name="pexp", tag="pexp")
                    sums = att_sb.tile([P, 4], F32, name="sums", tag="sums", bufs=2)
                    mx = att_sb.tile([P, 4], F32, name="mx", tag="mx", bufs=2)
                    small = att_sb.tile([P, 4], F32, name="small", tag="small", bufs=2)
                    negm1, negm, alpha, s_tot = (small[:, i:i + 1] for i in range(4))
                    nsl = slice(nb * P, (nb + 1) * P)

                    def score_half(lo_mch):
                        """2x 1024-wide PSUM tiles (4 key chunks); lhsT-outer
                        order so each stationary q slice loads once."""
                        sps = [sc_ps.tile([P, 1024], F32, name="sp", tag="sp")
                               for _ in range(2)]
                        for t in range(CT):
                            for j in range(4):
                                sp = sps[j // 2][:, (j % 2) * 512:(j % 2 + 1) * 512]
                                msl = slice((lo_mch + j) * 512, (lo_mch + j + 1) * 512)
                                nc.tensor.matmul(
                                    sp, q16[t][:, nsl], k16[t][:, msl],
                                    start=(t == 0), stop=(t == CT - 1))
                        return sps

                    # pass A: key chunks 0..3
                    spA = score_half(0)
                    for i in range(2):
                        nc.vector.reduce_max(out=mx[:, i:i + 1], in_=spA[i], axis=AX)
                    nc.vector.reduce_max(out=negm1, in_=mx[:, 0:2], axis=AX, negate=True)
                    for i in range(2):
                        nc.scalar.activation(
                            out=pt_b[:, i * 1024:(i + 1) * 1024], in_=spA[i],
                            func=ACTF.Exp, bias=negm1, scale=1.0,
                            accum_out=sums[:, i:i + 1])
                    return (pt_b, sums, mx, small, score_half, nsl)

                def emit_scores_b(stA):
                    """Pass B: scores chunks 4-7, combined max, rescale of A."""
                    pt_b, sums, mx, small, score_half, nsl = stA
                    negm1, negm, alpha, s_tot = (small[:, i:i + 1] for i in range(4))
                    # pass B: key chunks 4..7
                    spB = score_half(4)
                    for i in range(2):
                        nc.vector.reduce_max(out=mx[:, 2 + i:3 + i], in_=spB[i], axis=AX)
                    nc.vector.reduce_max(out=negm, in_=mx[:, 2:4], axis=AX, negate=True)
                    nc.vector.tensor_tensor(out=negm, in0=negm, in1=negm1, op=ALU.min)
                    nc.vector.tensor_tensor(out=alpha, in0=negm, in1=negm1, op=ALU.subtract)
                    nc.scalar.activation(out=alpha, in_=alpha, func=ACTF.Exp)
                    for i in range(2):
                        nc.scalar.activation(
                            out=pt_b[:, (2 + i) * 1024:(3 + i) * 1024], in_=spB[i],
                            func=ACTF.Exp, bias=negm, scale=1.0,
                            accum_out=sums[:, 2 + i:3 + i])
                    # rescale pass-A exp by alpha (in place, fp16 2x)
                    nc.vector.tensor_scalar_mul(
                        out=pt_b[:, 0:NQ], in0=pt_b[:, 0:NQ], scalar1=alpha)
                    return pt_b, sums, alpha, s_tot

                def emit_apply_1(nb, st):
                    """First half of transposes + attnV for block nb."""
                    po = o_ps.tile([P, C], F32, name="po", tag="po")
                    return po, self_apply_groups(nb, st, po, range(2))

                def self_apply_groups(nb, st, po, g2s):
                    pt_b, sums, alpha, s_tot = st
                    for g2 in g2s:
                        tp = tp_ps.tile([P, 1024], F16, name="tp", tag="tp")
                        for j in range(8):
                            mt = 8 * g2 + j
                            nc.tensor.transpose(
                                tp[:, j * P:(j + 1) * P], pt_b[:, mt * P:(mt + 1) * P], ident)
                        ptg = pt_pool.tile([P, 1024], F16, name="ptg", tag="ptg")
                        nc.vector.tensor_copy(ptg, tp)
                        for j in range(8):
                            mt = 8 * g2 + j
                            nc.tensor.matmul(
                                po, ptg[:, j * P:(j + 1) * P], vT[mt],
                                start=(mt == 0), stop=(mt == N // P - 1))

                def emit_apply_2(nb, st, po):
                    """Second half of transposes/attnV + normalize + out transpose."""
                    pt_b, sums, alpha, s_tot = st
                    nsl = slice(nb * P, (nb + 1) * P)
                    self_apply_groups(nb, st, po, range(2, 4))
                    # normalize: S = alpha*sum(A) + sum(B); out_T *= 1/S
                    nc.vector.tensor_scalar_mul(
                        out=sums[:, 0:2], in0=sums[:, 0:2], scalar1=alpha)
                    nc.vector.reduce_sum(out=s_tot, in_=sums, axis=AX)
                    recip = att_sb.tile([P, 1], F32, name="recip", tag="recip", bufs=2)
                    nc.vector.reciprocal(out=recip, in_=s_tot)
                    oT = ot_pool.tile([P, C], F16, name="oT", tag="oT")
                    nc.vector.tensor_scalar_mul(out=oT, in0=po, scalar1=recip)

                    # transpose out_T back to [c, n]
                    tp2 = tp_ps.tile([P, 512], F16, name="tp2", tag="tp")
                    for t in range(CT):
                        nc.tensor.transpose(
                            tp2[:, t * P:(t + 1) * P], oT[:, t * P:(t + 1) * P], ident)
                    tp2v = tp2.rearrange("p (t n) -> p t n", t=CT)
                    nc.vector.tensor_copy(out_ca[:, :, nsl], tp2v)

                # software pipeline: apply(nb-1) sits between pass A and pass B of
                # block nb so PE has guaranteed work while the pass-A softmax chain
                # (DVE max -> ACT exp) frees the score PSUM banks
                prev = None
                for nb in range(NQB + 1):
                    stA = emit_scores_a(nb) if nb < NQB else None
                    if prev is not None:
                        po_prev = emit_apply_1(nb - 1, prev)[0]
                    stB = emit_scores_b(stA) if nb < NQB else None
                    if prev is not None:
                        emit_apply_2(nb - 1, prev, po_prev)
                        if (nb - 1) % 4 == 3:
                            emit_proj((nb - 1) // 4)
                    prev = stB

    nc.compile()
    return nc


def _prep_shared(gn_w, gn_b, wq, bq, wk, bk, wv, bv, wp, bp):
    f32 = np.float32
    s = f32(math.sqrt(512.0))
    def pack(wT):  # [C, C] -> [P, CT, C] partition-major
        return np.ascontiguousarray(wT.reshape(CT, P, C).transpose(1, 0, 2))

    prm = np.zeros((P, CT, 6), dtype=f32)
    prm[:, :, 0] = (bq.astype(f32) * s).reshape(CT, P).T
    prm[:, :, 1] = bk.astype(f32).reshape(CT, P).T
    prm[:, :, 2] = bp.astype(f32).reshape(CT, P).T
    prm[:, :, 3] = gn_w.astype(f32).reshape(CT, P).T
    prm[:, :, 4] = gn_b.astype(f32).reshape(CT, P).T
    shared = {
        "wqt": pack((wq.T * s).astype(f32)).astype(np.float16),
        "wkt": pack(wk.T.astype(f32)).astype(np.float16),
        "wvt": pack(wv.T.astype(f32)).astype(np.float16),
        "wpt": pack(wp.T.astype(f32)).astype(np.float16),
        "prm": prm,
        "bv": bv.astype(f32).reshape(1, C).astype(np.float16),
    }
    return shared


def _make_in_maps(inputs):
    x = np.asarray(inputs["x"], dtype=np.float32)
    args = [np.asarray(inputs[k], dtype=np.float32) for k in
            ("gn_w", "gn_b", "wq", "bq", "wk", "bk", "wv", "bv", "wp", "bp")]
    shared = _prep_shared(*args)
    in_maps = []
    for core in range(8):
        b, half = core // 2, core % 2
        xb = x[b].reshape(C, N)
        if half:
            xb = np.concatenate([xb[:, NQ:], xb[:, :NQ]], axis=1)
        m = dict(shared)
        m["x"] = np.ascontiguousarray(xb.reshape(CT, P, N))
        in_maps.append(m)
    return in_maps


def kernel(x, gn_w, gn_b, wq, bq, wk, bk, wv, bv, wp, bp):
    global _CACHED_NC
    if _CACHED_NC is None:
        _CACHED_NC = build_nc()
    nc = _CACHED_NC

    in_maps = _make_in_maps(dict(x=x, gn_w=gn_w, gn_b=gn_b, wq=wq, bq=bq, wk=wk,
                                 bk=bk, wv=wv, bv=bv, wp=wp, bp=bp))
    res = run_bass_kernel_spmd(nc, in_maps, core_ids=list(range(8)))

    y = np.empty((B, C, N), dtype=np.float32)
    for core in range(8):
        b, half = core // 2, core % 2
        y[b][:, half * NQ:(half + 1) * NQ] = res.results[core]["out"].reshape(C, NQ)
    return y.reshape(B, C, H, W)


# revision 3
# speedup vs baseline: 1.1807x; 1.1807x over previous
"""AttnBlock kernel for 8 Trainium2 NeuronCores — single-pass fp16, v3.

Sharding: 8 cores = 4 examples x 2 query-halves (columns rotated per core so
its half comes first; softmax over keys is permutation invariant). No
cross-core communication.

v3 structure: x is DMA'd once (phase 1), stats via bn_stats, and an fp16 copy
of x is kept in SBUF for the convs and the residual (saves 16 MB of HBM
re-reads). Convs and scores are single-pass fp16 (validated rel_err ~1e-2 on
the fixed inputs). Attention uses a 2-half online softmax with chunk-contiguous
score matmuls (per-chunk maxes overlap the score stream), 5 rotating score
PSUM banks so pass B starts before all of pass A is consumed, interleaved
transpose/attnV groups, and the proj of each 4-block group spread across the
following iteration.
"""

import math

import numpy as np

import concourse.bacc as bacc
import concourse.mybir as mybir
import concourse.tile as tile
from concourse.bass_utils import run_bass_kernel_spmd
from concourse.masks import make_identity

F32 = mybir.dt.float32
F16 = mybir.dt.float16

B, C, H, W = 4, 512, 64, 64
N = H * W            # 4096 key positions
NQ = N // 2          # 2048 query positions per core
P = 128              # partitions
CT = C // P          # 4 channel tiles
NCH = N // 512       # 8 key chunks of 512
NQB = NQ // P        # 16 query blocks of 128
G = 2                # groupnorm groups
EPS = 1e-6
AX = mybir.AxisListType.X
ALU = mybir.AluOpType
ACTF = mybir.ActivationFunctionType

_CACHED_NC = None


def build_nc(loop_r: int = 1):
    nc = bacc.Bacc("TRN2", target_bir_lowering=False)

    x_d = nc.dram_tensor("x", [CT, P, N], F32, kind="ExternalInput")
    wqt_d = nc.dram_tensor("wqt", [P, CT, C], F16, kind="ExternalInput")  # [p, t, o], q scaled by sqrt(C)
    wkt_d = nc.dram_tensor("wkt", [P, CT, C], F16, kind="ExternalInput")
    wvt_d = nc.dram_tensor("wvt", [P, CT, C], F16, kind="ExternalInput")
    wpt_d = nc.dram_tensor("wpt", [P, CT, C], F16, kind="ExternalInput")
    # per-channel params packed: [p, t, (bq, bk, bp', gnw, gnb, pad)]
    # bp' = bp + wp @ bv  (v-bias folded into proj bias; attn rows sum to 1)
    prm_d = nc.dram_tensor("prm", [P, CT, 6], F32, kind="ExternalInput")
    out_d = nc.dram_tensor("out", [CT, P, NQ], F32, kind="ExternalOutput")

    import contextlib

    with tile.TileContext(nc) as tc:
        loop_ctx = tc.For_i(0, loop_r, 1) if loop_r > 1 else contextlib.nullcontext()
        with (
            loop_ctx,
            tc.tile_pool(name="singles", bufs=1) as singles,
            tc.tile_pool(name="persist", bufs=1) as persist,
            tc.tile_pool(name="h16_pool", bufs=8) as h16_pool,
        ):
            ident = singles.tile([P, P], F16, name="ident")
            make_identity(nc, ident)
            ones_f32 = singles.tile([P, P], F32, name="ones_f32")
            nc.vector.memset(ones_f32, 1.0)
            inv256 = singles.tile([P, 1], F32, name="inv256")
            nc.vector.memset(inv256, 1.0 / 256.0)
            eps_t = singles.tile([P, 1], F32, name="eps_t")
            nc.vector.memset(eps_t, EPS)

            # weights and per-channel params: one packed DMA each
            wqt_all = persist.tile([P, CT, C], F16, name="wqt_all")
            wkt_all = persist.tile([P, CT, C], F16, name="wkt_all")
            wvt_all = persist.tile([P, CT, C], F16, name="wvt_all")
            wpt_all = persist.tile([P, CT, C], F16, name="wpt_all")
            prm = persist.tile([P, CT, 6], F32, name="prm")
            # prm + k/q weights load before x (needed first); v/p weights are
            # queued on the sync queue behind the phase-1 x stream so they
            # don't steal HBM bandwidth from it
            nc.gpsimd.dma_start(out=prm, in_=prm_d[:, :, :])
            nc.gpsimd.dma_start(out=wkt_all, in_=wkt_d[:, :, :])
            nc.gpsimd.dma_start(out=wqt_all, in_=wqt_d[:, :, :])
            wqt = [wqt_all[:, t, :] for t in range(CT)]
            wkt = [wkt_all[:, t, :] for t in range(CT)]
            wvt = [wvt_all[:, t, :] for t in range(CT)]
            wpt = [wpt_all[:, t, :] for t in range(CT)]
            bq = [prm[:, t, 0:1] for t in range(CT)]
            bk = [prm[:, t, 1:2] for t in range(CT)]
            bp = [prm[:, t, 2:3] for t in range(CT)]
            gnw = [prm[:, t, 3:4] for t in range(CT)]
            gnb = [prm[:, t, 4:5] for t in range(CT)]

            # persistent activations (single-pass fp16)
            x16a = persist.tile([P, CT, N], F16, name="x16a")
            k16 = [persist.tile([P, N], F16, name=f"k16_{t}") for t in range(CT)]
            q16 = [persist.tile([P, NQ], F16, name=f"q16_{t}") for t in range(CT)]
            vTa = persist.tile([P, N // P, C], F16, name="vTa")
            vT = [vTa[:, m, :] for m in range(N // P)]
            out_ca = persist.tile([P, CT, NQ], F16, name="out_ca")
            out_c = [out_ca[:, t, :] for t in range(CT)]

            # ---------------- Phase 1: GroupNorm statistics + x16 ----------------
            with (
                tc.tile_pool(name="stat_sb", bufs=1) as stat_sb,
                tc.tile_pool(name="stat_ps", bufs=2, space="PSUM") as stat_ps,
            ):
                stats6 = [stat_sb.tile([P, NCH, 6], F32, name=f"st6_{t}") for t in range(CT)]
                for t in range(CT):
                    for hf in range(2):
                        xb = stat_sb.tile([P, N // 2], F32, name="xbig", tag="xbig", bufs=3)
                        nc.sync.dma_start(
                            out=xb, in_=x_d[t][:, hf * (N // 2):(hf + 1) * (N // 2)])
                        for c2 in range(NCH // 2):
                            ch = hf * (NCH // 2) + c2
                            nc.vector.bn_stats(
                                out=stats6[t][:, ch, :], in_=xb[:, c2 * 512:(c2 + 1) * 512])
                        # fp16 copy of x for convs + residual (ACT engine; DVE busy)
                        nc.scalar.activation(
                            out=x16a[:, t, hf * (N // 2):(hf + 1) * (N // 2)], in_=xb,
                            func=ACTF.Identity, scale=1.0)
                nc.sync.dma_start(out=wvt_all, in_=wvt_d[:, :, :])
                nc.sync.dma_start(out=wpt_all, in_=wpt_d[:, :, :])
                mvs = stat_sb.tile([P, CT, 2], F32, name="mvs")
                for t in range(CT):
                    nc.vector.bn_aggr(out=mvs[:, t, :], in_=stats6[t])
                # stats2 cols: [mean_t0..3 | ex2_t0..3]
                stats2 = stat_sb.tile([P, 8], F32, name="stats2")
                means = mvs[:, :, 0]
                vars_ = mvs[:, :, 1]
                nc.vector.tensor_copy(stats2[:, 0:4], means)
                nc.vector.tensor_tensor(out=stats2[:, 4:8], in0=means, in1=means, op=ALU.mult)
                nc.vector.tensor_tensor(out=stats2[:, 4:8], in0=stats2[:, 4:8], in1=vars_, op=ALU.add)
                # column sums / 256 -> [1, 8] on partition 0
                ps8 = stat_ps.tile([1, 8], F32, name="ps8")
                nc.tensor.matmul(ps8, inv256, stats2, start=True, stop=True)
                s8 = stat_sb.tile([1, 8], F32, name="s8")
                nc.vector.tensor_copy(s8, ps8)
                # per-group mean and E[x^2]: adjacent-pair sums
                gme = stat_sb.tile([1, 4], F32, name="gme")  # [mu_g0, mu_g1, e_g0, e_g1]
                s8v = s8.rearrange("p (f g two) -> p f g two", f=2, two=2)
                gmev = gme.rearrange("p (f g) -> p f g", f=2)
                nc.vector.tensor_tensor(
                    out=gmev[:, :, :], in0=s8v[:, :, :, 0], in1=s8v[:, :, :, 1], op=ALU.add)
                # broadcast to 128 partitions: [128, 4]
                psb = stat_ps.tile([P, 4], F32, name="psb")
                nc.tensor.matmul(psb, ones_f32[0:1, :], gme, start=True, stop=True)
                mu_e = stat_sb.tile([P, 4], F32, name="mu_e")
                nc.vector.tensor_copy(mu_e, psb)
                mu_bc = mu_e[:, 0:2]
                e_bc = mu_e[:, 2:4]
                var_bc = stat_sb.tile([P, 2], F32, name="var_bc")
                nc.vector.tensor_tensor(out=var_bc, in0=mu_bc, in1=mu_bc, op=ALU.mult)
                nc.vector.tensor_tensor(out=var_bc, in0=e_bc, in1=var_bc, op=ALU.subtract)
                sd = stat_sb.tile([P, 2], F32, name="sd")
                for g in range(G):
                    nc.scalar.activation(out=sd[:, g:g + 1], in_=var_bc[:, g:g + 1],
                                         func=ACTF.Sqrt, bias=eps_t, scale=1.0)
                rstd = stat_sb.tile([P, 2], F32, name="rstd")
                nc.vector.reciprocal(out=rstd, in_=sd)
                # per-channel-tile affine: h = a*x + b
                a_t = [persist.tile([P, 1], F32, name=f"a_t{t}") for t in range(CT)]
                b_t = [persist.tile([P, 1], F32, name=f"b_t{t}") for t in range(CT)]
                for t in range(CT):
                    g = t // 2
                    nc.vector.tensor_tensor(
                        out=a_t[t], in0=gnw[t], in1=rstd[:, g:g + 1], op=ALU.mult)
                    nc.vector.tensor_tensor(
                        out=b_t[t], in0=mu_bc[:, g:g + 1], in1=a_t[t], op=ALU.mult)
                    nc.vector.tensor_tensor(
                        out=b_t[t], in0=gnb[t], in1=b_t[t], op=ALU.subtract)

            # ---------------- Phase 2: h + q/k/v convs (from SBUF x16) ----------------
            with (
                tc.tile_pool(name="cq_ps", bufs=2, space="PSUM") as cq_ps,
                tc.tile_pool(name="ck_ps", bufs=2, space="PSUM") as ck_ps,
                tc.tile_pool(name="cv_ps", bufs=2, space="PSUM") as cv_ps,
            ):
                for ch in range(NCH):
                    sl = slice(ch * 512, (ch + 1) * 512)
                    h16 = []
                    for t in range(CT):
                        h16t = h16_pool.tile([P, 512], F16, name="h16", tag="h16")
                        nc.vector.tensor_scalar(
                            out=h16t, in0=x16a[:, t, sl], scalar1=a_t[t], scalar2=b_t[t],
                            op0=ALU.mult, op1=ALU.add)
                        h16.append(h16t)
                    # k conv (and q for first half): single-pass fp16
                    for o in range(CT):
                        osl = slice(o * P, (o + 1) * P)
                        kp = ck_ps.tile([P, 512], F32, name="kp", tag="kp")
                        for t in range(CT):
                            nc.tensor.matmul(
                                kp, wkt[t][:, osl], h16[t],
                                start=(t == 0), stop=(t == CT - 1))
                        nc.scalar.activation(
                            out=k16[o][:, sl], in_=kp, func=ACTF.Identity,
                            bias=bk[o], scale=1.0)
                        if ch < NCH // 2:
                            qp = cq_ps.tile([P, 512], F32, name="qp", tag="qp")
                            for t in range(CT):
                                nc.tensor.matmul(
                                    qp, wqt[t][:, osl], h16[t],
                                    start=(t == 0), stop=(t == CT - 1))
                            nc.scalar.activation(
                                out=q16[o][:, sl], in_=qp, func=ACTF.Identity,
                                bias=bq[o], scale=1.0)
                    # v conv, transposed output (ch 6,7 deferred into phase 3
                    # as PE cover for block 0's softmax chains)
                    if ch < 6:
                        for mb in range(4):
                            m = ch * 4 + mb
                            vp = cv_ps.tile([P, C], F32, name="vp", tag="vp")
                            for t in range(CT):
                                nc.tensor.matmul(
                                    vp, h16[t][:, mb * P:(mb + 1) * P], wvt[t],
                                    start=(t == 0), stop=(t == CT - 1))
                            nc.vector.tensor_copy(vT[m], vp)

            # ---------------- Phase 3: attention ----------------
            with (
                tc.tile_pool(name="att_sb", bufs=1) as att_sb,
                tc.tile_pool(name="p_pool", bufs=2) as p_pool,
                tc.tile_pool(name="pt_pool", bufs=2) as pt_pool,
                tc.tile_pool(name="ot_pool", bufs=2) as ot_pool,
                tc.tile_pool(name="sc_ps", bufs=5, space="PSUM") as sc_ps,
                tc.tile_pool(name="tp_ps", bufs=1, space="PSUM") as tp_ps,
                tc.tile_pool(name="o_ps", bufs=1, space="PSUM") as o_ps,
                tc.tile_pool(name="pp_ps", bufs=1, space="PSUM") as pp_ps,
                tc.tile_pool(name="fin_pool", bufs=3) as fin_pool,
            ):
                def emit_proj_part(nch, o, pool=None):
                    sl = slice(nch * 512, (nch + 1) * 512)
                    pp = (pool.tile([P, 512], F32, name="pp", tag="sp") if pool
                          else pp_ps.tile([P, 512], F32, name="pp", tag="pp"))
                    for t in range(CT):
                        nc.tensor.matmul(
                            pp, wpt[t][:, o * P:(o + 1) * P], out_c[t][:, sl],
                            start=(t == 0), stop=(t == CT - 1))
                    fin = fin_pool.tile([P, 512], F32, name="fin", tag="fin")
                    nc.scalar.activation(
                        out=fin, in_=pp, func=ACTF.Identity, bias=bp[o], scale=1.0)
                    nc.vector.tensor_tensor(
                        out=fin, in0=fin, in1=x16a[:, o, sl], op=ALU.add)
                    nc.gpsimd.dma_start(out=out_d[o][:, sl], in_=fin)

                def emit_scores_half(nb, half, st=None):
                    """One key half (4 chunks of 512), chunk-contiguous matmuls
                    with each chunk's max emitted right after its matmuls."""
                    if half == 0:
                        pt_b = p_pool.tile([P, N], F16, name="pexp", tag="pexp")
                        sums = att_sb.tile([P, 8], F32, name="sums", tag="sums", bufs=2)
                        mx = att_sb.tile([P, 8], F32, name="mx", tag="mx", bufs=2)
                        small = att_sb.tile([P, 4], F32, name="small", tag="small", bufs=2)
                    else:
                        pt_b, sums, mx, small = st
                    negm1, negm, alpha, s_tot = (small[:, i:i + 1] for i in range(4))
                    nsl = slice(nb * P, (nb + 1) * P)
                    sps = []
                    for j in range(4):
                        mch = 4 * half + j
                        msl = slice(mch * 512, (mch + 1) * 512)
                        sp = sc_ps.tile([P, 512], F32, name="sp", tag="sp")
                        for t in range(CT):
                            nc.tensor.matmul(
                                sp, q16[t][:, nsl], k16[t][:, msl],
                                start=(t == 0), stop=(t == CT - 1))
                        nc.vector.reduce_max(out=mx[:, mch:mch + 1], in_=sp, axis=AX)
                        sps.append(sp)
                    if half == 0:
                        nc.vector.reduce_max(out=negm1, in_=mx[:, 0:4], axis=AX, negate=True)
                        for j in range(4):
                            nc.scalar.activation(
                                out=pt_b[:, j * 512:(j + 1) * 512], in_=sps[j],
                                func=ACTF.Exp, bias=negm1, scale=1.0,
                                accum_out=sums[:, j:j + 1])
                        return (pt_b, sums, mx, small)
                    else:
                        nc.vector.reduce_max(out=negm, in_=mx[:, 4:8], axis=AX, negate=True)
                        nc.vector.tensor_tensor(out=negm, in0=negm, in1=negm1, op=ALU.min)
                        nc.vector.tensor_tensor(out=alpha, in0=negm, in1=negm1, op=ALU.subtract)
                        nc.scalar.activation(out=alpha, in_=alpha, func=ACTF.Exp)
                        for j in range(4):
                            nc.scalar.activation(
                                out=pt_b[:, (4 + j) * 512:(5 + j) * 512], in_=sps[j],
                                func=ACTF.Exp, bias=negm, scale=1.0,
                                accum_out=sums[:, 4 + j:5 + j])
                        return (pt_b, sums, mx, small)

                def emit_tail(st):
                    """Rescale A-half by alpha; total sum and reciprocal."""
                    pt_b, sums, mx, small = st
                    negm1, negm, alpha, s_tot = (small[:, i:i + 1] for i in range(4))
                    nc.vector.tensor_scalar_mul(
                        out=pt_b[:, 0:NQ], in0=pt_b[:, 0:NQ], scalar1=alpha)
                    nc.vector.tensor_scalar_mul(
                        out=sums[:, 0:4], in0=sums[:, 0:4], scalar1=alpha)
                    recip = att_sb.tile([P, 1], F32, name="recip", tag="recip", bufs=2)
                    nc.vector.reduce_sum(out=s_tot, in_=sums, axis=AX)
                    nc.vector.reciprocal(out=recip, in_=s_tot)
                    return recip

                def emit_v_deferred(ch):
                    """v-conv for one deferred chunk (h16 recomputed on DVE);
                    vp double-buffered through the po/pp banks."""
                    sl = slice(ch * 512, (ch + 1) * 512)
                    h16 = []
                    for t in range(CT):
                        h16t = h16_pool.tile([P, 512], F16, name="h16", tag="h16")
                        nc.vector.tensor_scalar(
                            out=h16t, in0=x16a[:, t, sl], scalar1=a_t[t], scalar2=b_t[t],
                            op0=ALU.mult, op1=ALU.add)
                        h16.append(h16t)
                    for mb in range(4):
                        m = ch * 4 + mb
                        pool, tg = (o_ps, "po") if mb % 2 else (pp_ps, "pp")
                        vp = pool.tile([P, C], F32, name="vpd", tag=tg)
                        for t in range(CT):
                            nc.tensor.matmul(
                                vp, h16[t][:, mb * P:(mb + 1) * P], wvt[t],
                                start=(t == 0), stop=(t == CT - 1))
                        nc.vector.tensor_copy(vT[m], vp)

                def emit_apply_half(nb, st, po, g2s, tpool=None):
                    """Transpose + attnV for two groups of 8 key tiles,
                    T/copy/V interleaved. Groups 2,3 (B-half of pt_b, which
                    needs no alpha rescale) run first so apply1 does not wait
                    on the previous iteration's rescale."""
                    pt_b = st[0]
                    for g2 in g2s:
                        tp = (tpool.tile([P, 1024], F16, name="tp", tag="sp")
                              if tpool else
                              tp_ps.tile([P, 1024], F16, name="tp", tag="tp"))
                        for j in range(8):
                            mt = 8 * g2 + j
                            nc.tensor.transpose(
                                tp[:, j * P:(j + 1) * P], pt_b[:, mt * P:(mt + 1) * P], ident)
                        ptg = pt_pool.tile([P, 1024], F16, name="ptg", tag="ptg")
                        nc.vector.tensor_copy(ptg, tp)
                        for j in range(8):
                            mt = 8 * g2 + j
                            nc.tensor.matmul(
                                po, ptg[:, j * P:(j + 1) * P], vT[mt],
                                start=(mt == 16), stop=(mt == 15))

                def emit_out(nb, po, recip):
                    """Normalize + transpose out_T back to [c, n]."""
                    nsl = slice(nb * P, (nb + 1) * P)
                    oT = ot_pool.tile([P, C], F16, name="oT", tag="oT")
                    nc.vector.tensor_scalar_mul(out=oT, in0=po, scalar1=recip)
                    tp2 = tp_ps.tile([P, 512], F16, name="tp2", tag="tp")
                    for t in range(CT):
                        nc.tensor.transpose(
                            tp2[:, t * P:(t + 1) * P], oT[:, t * P:(t + 1) * P], ident)
                    tp2v = tp2.rearrange("p (t n) -> p t n", t=CT)
                    nc.vector.tensor_copy(out_ca[:, :, nsl], tp2v)

                # software pipeline across iterations:
                #   iter nb: scoresA(nb) | apply1(nb-1) | scoresB(nb) | apply2(nb-1)
                #   proj for 4-block group g spread across iter 4g+5.
                prev = None         # (st, po, recip) of block nb-1
                for nb in range(NQB + 2):
                    pj = nb - 2     # proj group index source block
                    do_proj = pj >= 3 and (pj % 4) == 3
                    gp = (pj // 4) if do_proj else None
                    # the last proj group runs after scores are done: pipeline
                    # its psum through the freed score banks instead of pp_ps
                    pjpool = sc_ps if (do_proj and gp == NQ // 2048 * 4 - 1) else None
                    if do_proj:
                        emit_proj_part(gp, 0, pjpool)
                    stA = emit_scores_half(nb, 0) if nb < NQB else None
                    if nb == 0:
                        emit_v_deferred(6)
                    # drain phase: scores are done, pipeline the last apply's
                    # transposes through the freed score banks
                    tpool = sc_ps if nb - 1 >= NQB - 1 else None
                    if prev is not None:
                        st_p, recip_p = prev
                        po = o_ps.tile([P, C], F32, name="po", tag="po")
                        emit_apply_half(nb - 1, st_p, po, (2, 3), tpool)
                    if do_proj:
                        emit_proj_part(gp, 1, pjpool)
                    if nb < NQB:
                        stB = emit_scores_half(nb, 1, stA)
                    if nb == 0:
                        emit_v_deferred(7)
                    if do_proj:
                        emit_proj_part(gp, 2, pjpool)
                    if prev is not None:
                        emit_apply_half(nb - 1, st_p, po, (0, 1), tpool)
                        emit_out(nb - 1, po, recip_p)
                    if nb < NQB:
                        recip = emit_tail(stB)
                        prev = (stB, recip)
                    else:
                        prev = None
                    if do_proj:
                        emit_proj_part(gp, 3, pjpool)

    nc.compile()
    return nc


def _prep_shared(gn_w, gn_b, wq, bq, wk, bk, wv, bv, wp, bp):
    f32 = np.float32
    s = f32(math.sqrt(512.0))
    def pack(wT):  # [C, C] -> [P, CT, C] partition-major
        return np.ascontiguousarray(wT.reshape(CT, P, C).transpose(1, 0, 2))

    prm = np.zeros((P, CT, 6), dtype=f32)
    prm[:, :, 0] = (bq.astype(f32) * s).reshape(CT, P).T
    prm[:, :, 1] = bk.astype(f32).reshape(CT, P).T
    # v-bias folded into the proj bias: attn rows sum to 1, so
    # proj(attn@v + bv) = proj(attn@v0) + (wp@bv + bp)
    bp2 = bp.astype(f32) + wp.astype(f32) @ bv.astype(f32)
    prm[:, :, 2] = bp2.reshape(CT, P).T
    prm[:, :, 3] = gn_w.astype(f32).reshape(CT, P).T
    prm[:, :, 4] = gn_b.astype(f32).reshape(CT, P).T
    shared = {
        "wqt": pack((wq.T * s).astype(f32)).astype(np.float16),
        "wkt": pack(wk.T.astype(f32)).astype(np.float16),
        "wvt": pack(wv.T.astype(f32)).astype(np.float16),
        "wpt": pack(wp.T.astype(f32)).astype(np.float16),
        "prm": prm,
    }
    return shared


def _make_in_maps(inputs):
    x = np.asarray(inputs["x"], dtype=np.float32)
    args = [np.asarray(inputs[k], dtype=np.float32) for k in
            ("gn_w", "gn_b", "wq", "bq", "wk", "bk", "wv", "bv", "wp", "bp")]
    shared = _prep_shared(*args)
    in_maps = []
    for core in range(8):
        b, half = core // 2, core % 2
        xb = x[b].reshape(C, N)
        if half:
            xb = np.concatenate([xb[:, NQ:], xb[:, :NQ]], axis=1)
        m = dict(shared)
        m["x"] = np.ascontiguousarray(xb.reshape(CT, P, N))
        in_maps.append(m)
    return in_maps


def kernel(x, gn_w, gn_b, wq, bq, wk, bk, wv, bv, wp, bp):
    global _CACHED_NC
    if _CACHED_NC is None:
        _CACHED_NC = build_nc()
    nc = _CACHED_NC

    in_maps = _make_in_maps(dict(x=x, gn_w=gn_w, gn_b=gn_b, wq=wq, bq=bq, wk=wk,
                                 bk=bk, wv=wv, bv=bv, wp=wp, bp=bp))
    res = run_bass_kernel_spmd(nc, in_maps, core_ids=list(range(8)))

    y = np.empty((B, C, N), dtype=np.float32)
    for core in range(8):
        b, half = core // 2, core % 2
        y[b][:, half * NQ:(half + 1) * NQ] = res.results[core]["out"].reshape(C, NQ)
    return y.reshape(B, C, H, W)
